# revision 2
# baseline (speedup 1.0000x reference)
"""Trainium2 Bass kernel for nn_BasicNCAModel (neural cellular automaton).

Full inputs in, full outputs out. Sharding: pure data parallel over batch
(B=8 -> 8 NeuronCores, one image per core); all params replicated.

Per NCA step (per core, image x [20, 256, 256] fp32):
  perc   = concat([x, dwconv3x3_reflect(x, w_f1), dwconv3x3_reflect(x, w_f2)])
  h      = relu(W1 @ perc + b1)            # 60 -> 128, 1x1
  dx     = W2 @ h                          # 128 -> 20, 1x1
  x      = x + dx * stoch * ch_mask

Kernel formulation (all per-pixel work on the PE array):
  h_pre[o, p] = sum_{dy,dx,c} A[dx][dy*20+c, o] * x[c, py+dy, px+dx]  (reflect)
  with A[dx][g*20+c, o] = W1[o,20+c]*w1[c,dy+1,dx+1] + W1[o,40+c]*w2[c,dy+1,dx+1]
                          (+ W1[o,c] at dy=dx=0)
  This is 3 PSUM-accumulated matmuls (one per dx in {-1,0,1}) against a
  stacked SBUF tile S[61, R, W+2] holding 3 row-shifted copies of x
  (partitions 0-19: dy=0, 20-39: dy=-1, 40-59: dy=+1) plus a "penalty" row
  (partition 60, center matmul only) = -1e5 where the stochastic fire mask
  is 0, so relu(h_pre + pen) == 0 there and the update becomes exactly
  x' = x.  The channel mask is folded into W2 (rows 0-2 zeroed).
  Then: h = relu(h_pre + b1) on ScalarE (fused bias), dx = W2T @ h on PE,
  x' = x + dx on VectorE (reads the dy=0 group of S for x).

Matmuls run as float32r (full-rate on TRN2 for moving dim >= 256).
x ping-pongs through DRAM scratch tensors between steps; reflect padding is
implemented with split row DMAs and two small on-chip pad-column copies.

The stochastic mask replicates jax.random:  fold_in(key(42), step) ->
uniform(B,1,H,W) < 0.5, computed host-side on CPU jax (bit-exact vs the
reference since threefry is deterministic).
"""

import sys

if "/opt/trn_rl_repo" not in sys.path:
    sys.path.insert(0, "/opt/trn_rl_repo")

import numpy as np

C = 20
HID = 128
H = 256
W = 256
NCORES = 8
BIG = 1.0e5
FIRE_RATE = 0.5

_NC_CACHE = {}


def _build_nc(steps, n_loop=1, img_h=H, rows_per_tile=32):
    """Build + finalize the Bass module for `steps` NCA steps.

    n_loop > 1 wraps the whole steps-chain in a hardware For_i loop (each
    iteration recomputes from the input image) — used only for timing.
    """
    import concourse.bacc as bacc
    import concourse.tile as tile
    import concourse.mybir as mybir
    from contextlib import ExitStack

    f32 = mybir.dt.float32
    f32r = mybir.dt.float32r
    R = rows_per_tile
    assert img_h % R == 0 and R % 2 == 0

    nc = bacc.Bacc("TRN2", target_bir_lowering=False, debug=False,
                   num_devices=NCORES)
    x_in = nc.dram_tensor("x", [C, img_h, W], f32, kind="ExternalInput")
    taps_in = nc.dram_tensor("taps", [3, 61, HID], f32, kind="ExternalInput")
    w2t_in = nc.dram_tensor("w2t", [HID, C], f32, kind="ExternalInput")
    b1_in = nc.dram_tensor("b1", [HID, 1], f32, kind="ExternalInput")
    pen_in = nc.dram_tensor("pen", [steps, img_h, W], f32, kind="ExternalInput")
    y_out = nc.dram_tensor("y", [C, img_h, W], f32, kind="ExternalOutput")
    scr = [nc.dram_tensor(f"scr{i}", [C, img_h, W], f32, kind="Internal")
           for i in range(2)]

    with tile.TileContext(nc) as tc, ExitStack() as ctx:
        wpool = ctx.enter_context(tc.tile_pool(name="wpool", bufs=1))
        spool = ctx.enter_context(tc.tile_pool(name="spool", bufs=3))
        hpool = ctx.enter_context(tc.tile_pool(name="hpool", bufs=4))
        opool = ctx.enter_context(tc.tile_pool(name="opool", bufs=2))
        papool = ctx.enter_context(tc.tile_pool(name="papool", bufs=4, space="PSUM"))
        p2pool = ctx.enter_context(tc.tile_pool(name="p2pool", bufs=3, space="PSUM"))

        taps_sb = wpool.tile([61, 3, HID], f32r)
        nc.sync.dma_start(out=taps_sb, in_=taps_in.ap().rearrange("d k m -> k d m").bitcast(f32r))
        w2t_sb = wpool.tile([HID, C], f32r)
        nc.sync.dma_start(out=w2t_sb, in_=w2t_in[:, :].bitcast(f32r))
        b1_sb = wpool.tile([HID, 1], f32)
        nc.sync.dma_start(out=b1_sb, in_=b1_in[:, :])

        def emit_step(src, dst, s):
            for t in range(img_h // R):
                h0 = t * R
                S = spool.tile([61, R, W + 2], f32r, tag="S", name="S")
                # group 0 (partitions 0-19): dy=0 rows [h0, h0+R)
                nc.sync.dma_start(out=S[0:20, :, 1:W + 1], in_=src[:, h0:h0 + R, :].bitcast(f32r))
                # group 1 (partitions 20-39): dy=-1 rows, reflect at top
                if h0 == 0:
                    nc.sync.dma_start(out=S[20:40, 0:1, 1:W + 1], in_=src[:, 1:2, :].bitcast(f32r))
                    nc.sync.dma_start(out=S[20:40, 1:R, 1:W + 1],
                                      in_=src[:, 0:R - 1, :].bitcast(f32r))
                else:
                    nc.sync.dma_start(out=S[20:40, :, 1:W + 1],
                                      in_=src[:, h0 - 1:h0 + R - 1, :].bitcast(f32r))
                # group 2 (partitions 40-59): dy=+1 rows, reflect at bottom
                if h0 + R == img_h:
                    nc.sync.dma_start(out=S[40:60, 0:R - 1, 1:W + 1],
                                      in_=src[:, h0 + 1:img_h, :].bitcast(f32r))
                    nc.sync.dma_start(out=S[40:60, R - 1:R, 1:W + 1],
                                      in_=src[:, img_h - 2:img_h - 1, :].bitcast(f32r))
                else:
                    nc.sync.dma_start(out=S[40:60, :, 1:W + 1],
                                      in_=src[:, h0 + 1:h0 + R + 1, :].bitcast(f32r))
                # penalty row (partition 60), read by the center matmul only
                nc.sync.dma_start(out=S[60:61, :, 1:W + 1],
                                  in_=pen_in[s:s + 1, h0:h0 + R, :].bitcast(f32r))
                # reflect pad columns: S[.,.,0] = x[..,1], S[.,.,W+1] = x[..,W-2]
                nc.gpsimd.tensor_copy(out=S[0:60, :, 0:1], in_=S[0:60, :, 2:3])
                nc.gpsimd.tensor_copy(out=S[0:60, :, W + 1:W + 2],
                                      in_=S[0:60, :, W - 1:W])

                xo = opool.tile([C, R, W], f32, tag="xo", name="xo")
                for b in range(R // 2):
                    r0, r1 = 2 * b, 2 * b + 2
                    pa = papool.tile([HID, 2, W], f32, tag="pa", name="pa")
                    nc.tensor.matmul(pa, lhsT=taps_sb[0:60, 0, :],
                                     rhs=S[0:60, r0:r1, 0:W],
                                     start=True, stop=False)
                    nc.tensor.matmul(pa, lhsT=taps_sb[0:61, 1, :],
                                     rhs=S[0:61, r0:r1, 1:W + 1],
                                     start=False, stop=False)
                    nc.tensor.matmul(pa, lhsT=taps_sb[0:60, 2, :],
                                     rhs=S[0:60, r0:r1, 2:W + 2],
                                     start=False, stop=True)
                    ht = hpool.tile([HID, 2, W], f32r, tag="ht", name="ht")
                    nc.scalar.activation(out=ht, in_=pa,
                                         func=mybir.ActivationFunctionType.Relu,
                                         bias=b1_sb[:, 0:1], scale=1.0)
                    p2 = p2pool.tile([C, 2, W], f32, tag="p2", name="p2")
                    nc.tensor.matmul(p2, lhsT=w2t_sb[:, :],
                                     rhs=ht[:, :, :],
                                     start=True, stop=True)
                    nc.vector.tensor_tensor(out=xo[:, r0:r1, :], in0=p2,
                                            in1=S[0:20, r0:r1, 1:W + 1].bitcast(f32),
                                            op=mybir.AluOpType.add)
                nc.sync.dma_start(out=dst[:, h0:h0 + R, :], in_=xo)

        def emit_chain():
            if steps == 1:
                emit_step(x_in, y_out, 0)
                return
            emit_step(x_in, scr[0], 0)
            for s in range(1, steps - 1):
                emit_step(scr[(s - 1) % 2], scr[s % 2], s)
            emit_step(scr[(steps - 2) % 2], y_out, steps - 1)

        if n_loop == 1:
            emit_chain()
        else:
            with tc.For_i(0, n_loop):
                emit_chain()

    nc.finalize()
    return nc


def get_nc(steps, n_loop=1, img_h=H, rows_per_tile=32):
    key = (steps, n_loop, img_h, rows_per_tile)
    if key not in _NC_CACHE:
        _NC_CACHE[key] = _build_nc(steps, n_loop, img_h, rows_per_tile)
    return _NC_CACHE[key]


def _stoch_masks(steps, b, img_h=H):
    """Replicate reference RNG exactly: fold_in(key(42), step) -> uniform."""
    import jax

    cpu = jax.devices("cpu")[0]
    base_key = jax.random.key(42)
    out = np.empty((steps, b, img_h, W), dtype=bool)
    with jax.default_device(cpu):
        for s in range(steps):
            k = jax.random.fold_in(base_key, s)
            u = jax.random.uniform(k, (b, 1, img_h, W))
            out[s] = np.asarray(u[:, 0]) < FIRE_RATE
    return out


def make_host_inputs(x, w_f1, w_f2, W1, b1, W2, steps):
    """Precompute per-core DRAM inputs (taps lhsT, masked W2T, penalties)."""
    bsz = x.shape[0]
    A = np.zeros((3, 61, HID), np.float32)
    dys = (0, -1, 1)
    for j in range(3):
        for g, dy in enumerate(dys):
            c1 = w_f1[:, 0, dy + 1, j]
            c2 = w_f2[:, 0, dy + 1, j]
            A[j, 20 * g:20 * g + 20, :] = (W1[:, 20:40] * c1[None, :]).T \
                + (W1[:, 40:60] * c2[None, :]).T
            if dy == 0 and j == 1:
                A[j, 0:20, :] += W1[:, 0:20].T
    A[1, 60, :] = 1.0

    w2m = W2.copy()
    w2m[0:3, :] = 0.0
    w2t = np.ascontiguousarray(w2m.T).astype(np.float32)
    b1c = np.ascontiguousarray(b1.reshape(HID, 1)).astype(np.float32)

    stoch = _stoch_masks(steps, bsz, x.shape[2])
    pen = np.where(stoch, np.float32(0.0), np.float32(-BIG))  # [steps,B,H,W]

    in_maps = []
    for i in range(bsz):
        in_maps.append({
            "x": np.ascontiguousarray(x[i]).astype(np.float32),
            "taps": A,
            "w2t": w2t,
            "b1": b1c,
            "pen": np.ascontiguousarray(pen[:, i]),
        })
    return in_maps


def kernel(x, w_f1, w_f2, W1, b1, W2, steps):
    x = np.asarray(x, dtype=np.float32)
    w_f1 = np.asarray(w_f1, dtype=np.float32)
    w_f2 = np.asarray(w_f2, dtype=np.float32)
    W1 = np.asarray(W1, dtype=np.float32)
    b1 = np.asarray(b1, dtype=np.float32)
    W2 = np.asarray(W2, dtype=np.float32)
    steps = int(steps)
    if steps <= 0:
        return x.copy()

    from concourse.bass_utils import run_bass_kernel_spmd

    nc = get_nc(steps)
    in_maps = make_host_inputs(x, w_f1, w_f2, W1, b1, W2, steps)
    res = run_bass_kernel_spmd(nc, in_maps, core_ids=list(range(x.shape[0])))
    out = np.stack([res.results[i]["y"] for i in range(x.shape[0])], axis=0)
    return out.astype(np.float32)


# revision 4
# speedup vs baseline: 4.8577x; 4.8577x over previous
"""Trainium2 Bass kernel for nn_BasicNCAModel (neural cellular automaton).

Full inputs in, full outputs out. Sharding: pure data parallel over batch
(B=8 -> 8 NeuronCores, one image per core); all params replicated.

Per NCA step (per core, image x [20, 256, 256] fp32):
  perc   = concat([x, dwconv3x3_reflect(x, w_f1), dwconv3x3_reflect(x, w_f2)])
  h      = relu(W1 @ perc + b1)            # 60 -> 128, 1x1
  dx     = W2 @ h                          # 128 -> 20, 1x1
  x      = x + dx * stoch * ch_mask

Kernel formulation (all per-pixel work on the PE array):
  h_pre[o, p] = sum_{dy,dx,c} A[dx][dy*20+c, o] * x[c, py+dy, px+dx]  (reflect)
  with A[dx][g*20+c, o] = W1[o,20+c]*w1[c,dy+1,dx+1] + W1[o,40+c]*w2[c,dy+1,dx+1]
                          (+ W1[o,c] at dy=dx=0)
  This is 3 PSUM-accumulated matmuls (one per dx in {-1,0,1}) against a
  stacked SBUF tile S[61, R, W+2] holding 3 row-shifted copies of x
  (partitions 0-19: dy=0, 20-39: dy=-1, 40-59: dy=+1) plus a "penalty" row
  (partition 60, center matmul only) = -1e5 where the stochastic fire mask
  is 0, so relu(h_pre + pen) == 0 there and the update becomes exactly
  x' = x.  The channel mask is folded into W2 (rows 0-2 zeroed).
  Then: h = relu(h_pre + b1) on ScalarE (fused bias), dx = W2T @ h on PE,
  x' = x + dx on VectorE (reads the dy=0 group of S for x).

Matmuls run as float32r (full-rate on TRN2 for moving dim >= 256).
x ping-pongs through DRAM scratch tensors between steps; reflect padding is
implemented with split row DMAs and two small on-chip pad-column copies.

The stochastic mask replicates jax.random:  fold_in(key(42), step) ->
uniform(B,1,H,W) < 0.5, computed host-side on CPU jax (bit-exact vs the
reference since threefry is deterministic).
"""

import sys

if "/opt/trn_rl_repo" not in sys.path:
    sys.path.insert(0, "/opt/trn_rl_repo")

import numpy as np

C = 20
HID = 128
H = 256
W = 256
NCORES = 8
BIG = 1.0e5
FIRE_RATE = 0.5

_NC_CACHE = {}


def _build_nc(steps, n_loop=1, img_h=H, rows_per_tile=32):
    """Build + finalize the Bass module for `steps` NCA steps.

    n_loop > 1 wraps the whole steps-chain in a hardware For_i loop (each
    iteration recomputes from the input image) — used only for timing.
    """
    import concourse.bacc as bacc
    import concourse.tile as tile
    import concourse.mybir as mybir
    from contextlib import ExitStack

    f32 = mybir.dt.float32
    f32r = mybir.dt.float32r
    R = rows_per_tile
    assert img_h % R == 0 and R % 2 == 0

    nc = bacc.Bacc("TRN2", target_bir_lowering=False, debug=False,
                   num_devices=NCORES)
    x_in = nc.dram_tensor("x", [C, img_h, W], f32, kind="ExternalInput")
    taps_in = nc.dram_tensor("taps", [3, 61, HID], f32, kind="ExternalInput")
    w2t_in = nc.dram_tensor("w2t", [HID, C], f32, kind="ExternalInput")
    b1_in = nc.dram_tensor("b1", [HID, 1], f32, kind="ExternalInput")
    pen_in = nc.dram_tensor("pen", [steps, img_h, W], f32, kind="ExternalInput")
    y_out = nc.dram_tensor("y", [C, img_h, W], f32, kind="ExternalOutput")
    scr = [nc.dram_tensor(f"scr{i}", [C, img_h, W], f32, kind="Internal")
           for i in range(2)]

    with tile.TileContext(nc) as tc, ExitStack() as ctx:
        wpool = ctx.enter_context(tc.tile_pool(name="wpool", bufs=1))
        spool = ctx.enter_context(tc.tile_pool(name="spool", bufs=3))
        hpool = ctx.enter_context(tc.tile_pool(name="hpool", bufs=4))
        opool = ctx.enter_context(tc.tile_pool(name="opool", bufs=2))
        papool = ctx.enter_context(tc.tile_pool(name="papool", bufs=4, space="PSUM"))
        p2pool = ctx.enter_context(tc.tile_pool(name="p2pool", bufs=3, space="PSUM"))

        taps_sb = wpool.tile([61, 3, HID], f32r)
        nc.sync.dma_start(out=taps_sb, in_=taps_in.ap().rearrange("d k m -> k d m").bitcast(f32r))
        w2t_sb = wpool.tile([HID, C], f32r)
        nc.sync.dma_start(out=w2t_sb, in_=w2t_in[:, :].bitcast(f32r))
        b1_sb = wpool.tile([HID, 1], f32)
        nc.sync.dma_start(out=b1_sb, in_=b1_in[:, :])

        def emit_step(src, dst, s):
            # Software-pipelined emission: the PE stream must not contain
            # mm2(g) right after taps(g) — it would stall waiting for the
            # ScalarE relu of the same group.  Delay mm2/add of group g until
            # after taps of group g+DELAY have been issued.
            DELAY = 3
            pend = []

            def flush_one():
                ht, p2s, xo_t, r0, r1, wb = pend.pop(0)
                p2 = p2pool.tile([C, 2, W], f32, tag="p2", name="p2")
                nc.tensor.matmul(p2, lhsT=w2t_sb[:, :], rhs=ht[:, :, :],
                                 start=True, stop=True)
                nc.vector.tensor_tensor(out=xo_t[:, r0:r1, :], in0=p2,
                                        in1=p2s.bitcast(f32),
                                        op=mybir.AluOpType.add)
                if wb is not None:
                    wb()

            for t in range(img_h // R):
                h0 = t * R
                S = spool.tile([61, R, W + 2], f32r, tag="S", name="S")
                # group 0 (partitions 0-19): dy=0 rows [h0, h0+R)
                nc.sync.dma_start(out=S[0:20, :, 1:W + 1], in_=src[:, h0:h0 + R, :].bitcast(f32r))
                # group 1 (partitions 20-39): dy=-1 rows, reflect at top
                if h0 == 0:
                    nc.sync.dma_start(out=S[20:40, 0:1, 1:W + 1], in_=src[:, 1:2, :].bitcast(f32r))
                    nc.sync.dma_start(out=S[20:40, 1:R, 1:W + 1],
                                      in_=src[:, 0:R - 1, :].bitcast(f32r))
                else:
                    nc.sync.dma_start(out=S[20:40, :, 1:W + 1],
                                      in_=src[:, h0 - 1:h0 + R - 1, :].bitcast(f32r))
                # group 2 (partitions 40-59): dy=+1 rows, reflect at bottom
                if h0 + R == img_h:
                    nc.sync.dma_start(out=S[40:60, 0:R - 1, 1:W + 1],
                                      in_=src[:, h0 + 1:img_h, :].bitcast(f32r))
                    nc.sync.dma_start(out=S[40:60, R - 1:R, 1:W + 1],
                                      in_=src[:, img_h - 2:img_h - 1, :].bitcast(f32r))
                else:
                    nc.sync.dma_start(out=S[40:60, :, 1:W + 1],
                                      in_=src[:, h0 + 1:h0 + R + 1, :].bitcast(f32r))
                # penalty row (partition 60), read by the center matmul only
                nc.sync.dma_start(out=S[60:61, :, 1:W + 1],
                                  in_=pen_in[s:s + 1, h0:h0 + R, :].bitcast(f32r))
                # reflect pad columns: S[.,.,0] = x[..,1], S[.,.,W+1] = x[..,W-2]
                nc.gpsimd.tensor_copy(out=S[0:60, :, 0:1], in_=S[0:60, :, 2:3])
                nc.gpsimd.tensor_copy(out=S[0:60, :, W + 1:W + 2],
                                      in_=S[0:60, :, W - 1:W])

                xo = opool.tile([C, R, W], f32, tag="xo", name="xo")
                ngroups = R // 2
                for b in range(ngroups):
                    r0, r1 = 2 * b, 2 * b + 2
                    pa = papool.tile([HID, 2, W], f32, tag="pa", name="pa")
                    nc.tensor.matmul(pa, lhsT=taps_sb[0:60, 0, :],
                                     rhs=S[0:60, r0:r1, 0:W],
                                     start=True, stop=False)
                    nc.tensor.matmul(pa, lhsT=taps_sb[0:61, 1, :],
                                     rhs=S[0:61, r0:r1, 1:W + 1],
                                     start=False, stop=False)
                    nc.tensor.matmul(pa, lhsT=taps_sb[0:60, 2, :],
                                     rhs=S[0:60, r0:r1, 2:W + 2],
                                     start=False, stop=True)
                    ht = hpool.tile([HID, 2, W], f32r, tag="ht", name="ht")
                    nc.scalar.activation(out=ht, in_=pa,
                                         func=mybir.ActivationFunctionType.Relu,
                                         bias=b1_sb[:, 0:1], scale=1.0)
                    wb = None
                    if b == ngroups - 1:
                        def wb(dst=dst, h0=h0, xo=xo):
                            nc.sync.dma_start(out=dst[:, h0:h0 + R, :], in_=xo)
                    pend.append((ht, S[0:20, r0:r1, 1:W + 1], xo, r0, r1, wb))
                    while len(pend) > DELAY:
                        flush_one()
            while pend:
                flush_one()

        def emit_chain():
            if steps == 1:
                emit_step(x_in, y_out, 0)
                return
            emit_step(x_in, scr[0], 0)
            for s in range(1, steps - 1):
                emit_step(scr[(s - 1) % 2], scr[s % 2], s)
            emit_step(scr[(steps - 2) % 2], y_out, steps - 1)

        if n_loop == 1:
            emit_chain()
        else:
            with tc.For_i(0, n_loop):
                emit_chain()

    nc.finalize()
    return nc


def get_nc(steps, n_loop=1, img_h=H, rows_per_tile=32):
    key = (steps, n_loop, img_h, rows_per_tile)
    if key not in _NC_CACHE:
        _NC_CACHE[key] = _build_nc(steps, n_loop, img_h, rows_per_tile)
    return _NC_CACHE[key]


def _stoch_masks(steps, b, img_h=H):
    """Replicate reference RNG exactly: fold_in(key(42), step) -> uniform."""
    import jax

    cpu = jax.devices("cpu")[0]
    base_key = jax.random.key(42)
    out = np.empty((steps, b, img_h, W), dtype=bool)
    with jax.default_device(cpu):
        for s in range(steps):
            k = jax.random.fold_in(base_key, s)
            u = jax.random.uniform(k, (b, 1, img_h, W))
            out[s] = np.asarray(u[:, 0]) < FIRE_RATE
    return out


def make_host_inputs(x, w_f1, w_f2, W1, b1, W2, steps):
    """Precompute per-core DRAM inputs (taps lhsT, masked W2T, penalties)."""
    bsz = x.shape[0]
    A = np.zeros((3, 61, HID), np.float32)
    dys = (0, -1, 1)
    for j in range(3):
        for g, dy in enumerate(dys):
            c1 = w_f1[:, 0, dy + 1, j]
            c2 = w_f2[:, 0, dy + 1, j]
            A[j, 20 * g:20 * g + 20, :] = (W1[:, 20:40] * c1[None, :]).T \
                + (W1[:, 40:60] * c2[None, :]).T
            if dy == 0 and j == 1:
                A[j, 0:20, :] += W1[:, 0:20].T
    A[1, 60, :] = 1.0

    w2m = W2.copy()
    w2m[0:3, :] = 0.0
    w2t = np.ascontiguousarray(w2m.T).astype(np.float32)
    b1c = np.ascontiguousarray(b1.reshape(HID, 1)).astype(np.float32)

    stoch = _stoch_masks(steps, bsz, x.shape[2])
    pen = np.where(stoch, np.float32(0.0), np.float32(-BIG))  # [steps,B,H,W]

    in_maps = []
    for i in range(bsz):
        in_maps.append({
            "x": np.ascontiguousarray(x[i]).astype(np.float32),
            "taps": A,
            "w2t": w2t,
            "b1": b1c,
            "pen": np.ascontiguousarray(pen[:, i]),
        })
    return in_maps


def kernel(x, w_f1, w_f2, W1, b1, W2, steps):
    x = np.asarray(x, dtype=np.float32)
    w_f1 = np.asarray(w_f1, dtype=np.float32)
    w_f2 = np.asarray(w_f2, dtype=np.float32)
    W1 = np.asarray(W1, dtype=np.float32)
    b1 = np.asarray(b1, dtype=np.float32)
    W2 = np.asarray(W2, dtype=np.float32)
    steps = int(steps)
    if steps <= 0:
        return x.copy()

    from concourse.bass_utils import run_bass_kernel_spmd

    nc = get_nc(steps)
    in_maps = make_host_inputs(x, w_f1, w_f2, W1, b1, W2, steps)
    res = run_bass_kernel_spmd(nc, in_maps, core_ids=list(range(x.shape[0])))
    out = np.stack([res.results[i]["y"] for i in range(x.shape[0])], axis=0)
    return out.astype(np.float32)
